# revision 40
# baseline (speedup 1.0000x reference)
"""Trainium2 Bass kernel: GAT-style attention layer, data-parallel over 8 NeuronCores.

Reference computation (per node n, K=32 neighbors, D=128 features, L=64 labels):
    h     = lrelu(x @ W)                  [N,K,D]
    e     = lrelu(h @ v + bias)           [N,K,1]
    alpha = softmax_k(e)                  [N,K]
    out   = sum_k alpha[n,k] * labels[n,k,:]   [N,L]

Sharding: pure data parallel over nodes (6250/core, zero-padded to 6400).
Host side re-lays each core's shard out so every DMA is contiguous per
partition; values are untouched (fp32 in DRAM).

Device pipeline per 256-node tile (software-pipelined by one tile so the
TensorE never idles and its HAM clock stays at 2.4 GHz):
  mm1   z^T[e,(k,n)] = W^T @ x^T          TensorE bf16, PSUM f32
  lrelu PSUM->SBUF bf16 (ScalarE Prelu, fused activation)
  mm2   s[k,n] = v^T @ h^T via selector weights, PSUM-accumulated rows
  e=lrelu(s+bias), w=exp(e)               ScalarE (bias is per-partition AP)
  w^T   TensorE transpose -> [n,k]; row sums via ScalarE accum_out
  alpha = w^T * (1/sums)                  VectorE (per-partition scalar)
  diag(alpha_k) tiles on VectorE (identity mask * scalar)
  agg   out^T = sum_k labels_k^T-as-stationary @ diag(alpha_k)
        PSUM-accumulated; interleaved into the NEXT tile's mm1 stream
"""
import sys

sys.path.insert(0, "/opt/trn_rl_repo")
import numpy as np

N, K, D, L = 50000, 32, 128, 64
NEG = 0.2
NCORES = 8
NPER = N // NCORES          # 6250
TN = 256                    # nodes per tile
NSUB = TN // 128            # sub-tiles of 128 nodes
NPAD = 6400                 # padded nodes per core
NT = NPAD // TN             # 25 tiles

LAST_RESULT = None
_cache = {}


def build(nt):
    import concourse.bass as bass
    import concourse.tile as tile
    from concourse import bacc, mybir

    f32 = mybir.dt.float32
    bf16 = mybir.dt.bfloat16
    AF = mybir.ActivationFunctionType
    OP = mybir.AluOpType
    PSUM = bass.MemorySpace.PSUM

    nc = bacc.Bacc(
        "TRN2", target_bir_lowering=False, debug=False, num_devices=NCORES
    )
    x_ext = nc.declare_dram_parameter("x", [nt, 128, K * TN], f32, False)
    lab_ext = nc.declare_dram_parameter("lab", [nt, 128, NSUB * K * L], f32, False)
    w_ext = nc.declare_dram_parameter("w", [D, D], f32, False)
    v_ext = nc.declare_dram_parameter("v", [D, 1], f32, False)
    b_ext = nc.declare_dram_parameter("b", [K, 1], f32, False)
    out_ext = nc.declare_dram_parameter("out", [nt, 128, NSUB * L], f32, isOutput=True)

    with tile.TileContext(nc) as tc:
        with (
            tc.tile_pool(name="const", bufs=1) as const,
            tc.tile_pool(name="xp", bufs=3) as xp,
            tc.tile_pool(name="labp", bufs=3) as labp,
            tc.tile_pool(name="hp", bufs=2) as hp,
            tc.tile_pool(name="wp", bufs=2) as wp,
            tc.tile_pool(name="smallp", bufs=4) as smallp,
            tc.tile_pool(name="dkp", bufs=2) as dkp,
            tc.tile_pool(name="outp", bufs=2) as outp,
            tc.tile_pool(name="zps", bufs=4, space=PSUM) as zps,
            tc.tile_pool(name="sps", bufs=2, space=PSUM) as sps,
            tc.tile_pool(name="wtps", bufs=1, space=PSUM) as wtps,
        ):
            W_sb = const.tile([128, 128], bf16)
            nc.gpsimd.dma_start(W_sb[:], w_ext[:])      # f32 -> bf16 cast DMA
            v_sb = const.tile([128, 1], bf16)
            nc.gpsimd.dma_start(v_sb[:], v_ext[:])
            bias_sb = const.tile([32, 1], f32)
            nc.sync.dma_start(bias_sb[:], b_ext[:])
            ones = const.tile([128, 128], bf16)
            nc.vector.memset(ones[:], 1.0)
            mask = const.tile([128, 128], bf16)         # identity matrix
            nc.gpsimd.affine_select(
                mask[:], ones[:], pattern=[[1, 128]],
                compare_op=OP.is_equal, fill=0.0, base=0, channel_multiplier=-1,
            )
            # vks[:, 32k+m] = v * (m == k): selector weights so score matmul k
            # writes only PSUM row k of a [32, TN] tile (base partition stays 0)
            vks = const.tile([128, K * 32], bf16)
            nc.vector.memset(vks[:], 0.0)
            nc.vector.tensor_copy(
                vks[:, 0:K * 32:33], v_sb[:, 0:1].broadcast_to([128, 32])
            )

            # PE warmup burst: dense dummy matmuls while the first x tile
            # loads, so the HAM clock gate opens before real work starts
            warm_ps = zps.tile([128, 512], f32, name="warm_ps", tag="warm", bufs=1)
            for _ in range(20):
                nc.tensor.matmul(
                    warm_ps[:], W_sb[:], vks[:, 0:512], skip_group_check=True
                )

            nchunk = (K * TN) // 512     # 16 mm1 chunks per tile
            prev = None                  # state of tile t-1 awaiting aggregation

            def emit_softmax_tail(st):
                """TensorE transpose of exp-weights to [node, k], row sums via
                ScalarE accum_out, then normalized alpha (bf16) per sub-tile."""
                w_sb = st["w_sb"]
                st["alphaN"] = []
                for s in range(NSUB):
                    wT_ps = wtps.tile([128, 32], bf16)
                    nc.tensor.transpose(
                        wT_ps[:], w_sb[:, s * 128:(s + 1) * 128], mask[0:32, 0:32]
                    )
                    wT_sb = smallp.tile([128, 32], f32)
                    sums = smallp.tile([128, 1], f32)
                    nc.scalar.activation(wT_sb[:], wT_ps[:], AF.Copy, accum_out=sums[:])
                    recip = smallp.tile([128, 1], f32)
                    nc.vector.reciprocal(recip[:], sums[:])
                    alphaN = smallp.tile([128, 32], bf16, name=f"alphaN{s}", tag=f"al{s}")
                    nc.vector.tensor_scalar_mul(alphaN[:], wT_sb[:], recip[:, 0:1])
                    st["alphaN"].append(alphaN)
                st["out_sb"] = outp.tile([128, NSUB * L], f32, name="out_sb", tag="out")

            def emit_agg(st, s):
                """Weighted label aggregation for sub-tile s of tile st on
                VectorE: prod = labels * alpha (broadcast over l), then a
                strided reduction over k."""
                lab3 = st["lab_sb"][:, s * K * L:(s + 1) * K * L].rearrange(
                    "p (k l) -> p k l", k=K
                )
                al3 = st["alphaN"][s][:, 0:K].rearrange(
                    "p (k o) -> p k o", o=1
                ).broadcast_to([128, K, L])
                prod = dkp.tile([128, K * L], bf16, name=f"prod{s}", tag=f"prod{s}")
                nc.vector.tensor_tensor(
                    prod[:].rearrange("p (k l) -> p k l", k=K), lab3, al3, OP.mult
                )
                nc.vector.tensor_reduce(
                    st["out_sb"][:, s * L:(s + 1) * L],
                    prod[:].rearrange("p (k l) -> p l k", k=K),
                    op=OP.add, axis=mybir.AxisListType.X,
                )

            def emit_agg_finish(st):
                nc.sync.dma_start(out_ext[st["t"]], st["out_sb"][:])

            for t in range(nt):
                x_sb = xp.tile([128, K * TN], bf16)
                half = K * TN // 2
                nc.gpsimd.dma_start(x_sb[:, 0:half], x_ext[t][:, 0:half])
                nc.gpsimd.dma_start(x_sb[:, half:], x_ext[t][:, half:])
                # labels stay f32 and ride the HWDGE path: keeps the big cast
                # loads (x) alone on SWDGE, easing Q7 descriptor-ring pressure
                lab_sb = labp.tile([128, NSUB * K * L], f32)
                lhalf = NSUB * K * L // 2
                nc.sync.dma_start(lab_sb[:, 0:lhalf], lab_ext[t][:, 0:lhalf])
                nc.sync.dma_start(lab_sb[:, lhalf:], lab_ext[t][:, lhalf:])

                h_sb = hp.tile([128, K * TN], bf16)
                # two interleaved score-accumulation chains on alternating
                # PSUM banks (avoids same-bank drain turnaround between
                # consecutive accumulating matmuls)
                s_psA = sps.tile([32, TN], f32, name="s_psA", tag="spsA", bufs=1)
                s_psB = sps.tile([32, TN], f32, name="s_psB", tag="spsB", bufs=1)

                def emit_mm2(k):
                    tgt = s_psA if k % 2 == 0 else s_psB
                    nc.tensor.matmul(
                        tgt[:], vks[:, k * 32:(k + 1) * 32],
                        h_sb[:, k * TN:(k + 1) * TN],
                        start=(k < 2), stop=(k >= K - 2),
                    )

                # chunk 0, then the previous tile's softmax tail (transpose on
                # PE + diag builds on DVE) so its agg matmuls are ready to
                # interleave into the remaining chunks. mm2 for chunk c runs
                # one chunk late so the ScalarE lrelu latency is hidden.
                z_ps = zps.tile([128, 512], f32)
                nc.tensor.matmul(z_ps[:], W_sb[:], x_sb[:, 0:512])
                nc.scalar.activation(h_sb[:, 0:512], z_ps[:], AF.Prelu, alpha=NEG)
                if prev is not None:
                    emit_softmax_tail(prev)
                for c in range(1, nchunk):
                    z_ps = zps.tile([128, 512], f32)
                    nc.tensor.matmul(z_ps[:], W_sb[:], x_sb[:, c * 512:(c + 1) * 512])
                    nc.scalar.activation(
                        h_sb[:, c * 512:(c + 1) * 512], z_ps[:], AF.Prelu, alpha=NEG
                    )
                    if prev is not None and c == 4:
                        emit_agg(prev, 0)
                    if prev is not None and c == 9:
                        emit_agg(prev, 1)
                if prev is not None:
                    emit_agg_finish(prev)
                for k in range(K):
                    emit_mm2(k)

                s_tmp = wp.tile([32, TN], f32)
                nc.vector.tensor_copy(s_tmp[:], s_psA[:])
                s_sum = wp.tile([32, TN], f32)
                nc.vector.tensor_add(s_sum[:], s_tmp[:], s_psB[:])
                e_sb = wp.tile([32, TN], f32)
                nc.scalar.activation(
                    e_sb[:], s_sum[:], AF.Prelu, bias=bias_sb[:, 0:1], alpha=NEG
                )
                w_sb = wp.tile([32, TN], bf16)
                nc.scalar.activation(w_sb[:], e_sb[:], AF.Exp)

                prev = {"t": t, "w_sb": w_sb, "lab_sb": lab_sb}

            # drain the last tile
            emit_softmax_tail(prev)
            emit_agg(prev, 0)
            emit_agg(prev, 1)
            emit_agg_finish(prev)
    nc.compile()
    return nc


def shard_inputs(x, lab, nt=NT, nper=NPER):
    npad = nt * TN
    xs = np.zeros((npad, K, D), np.float32)
    xs[:nper] = x
    ls = np.zeros((npad, K, L), np.float32)
    ls[:nper] = lab
    xf = np.ascontiguousarray(
        xs.reshape(nt, TN, K, D).transpose(0, 3, 2, 1)
    ).reshape(nt, 128, K * TN)
    lf = np.ascontiguousarray(
        ls.reshape(nt, NSUB, 128, K * L).transpose(0, 2, 1, 3)
    ).reshape(nt, 128, NSUB * K * L)
    return xf, lf


def unshard_output(o, nt=NT, nper=NPER):
    # o[t, p, s*L + l] = pred[node = t*TN + s*128 + p, l]
    return (
        o.reshape(nt, 128, NSUB, L).transpose(0, 2, 1, 3).reshape(nt * TN, L)[:nper]
    )


def kernel(para_neighbors, para_nei_labels, linear, e_vec, bias):
    from concourse.bass_utils import run_bass_kernel_spmd

    global LAST_RESULT
    x = np.asarray(para_neighbors, np.float32)
    lab = np.asarray(para_nei_labels, np.float32)
    Wm = np.ascontiguousarray(np.asarray(linear, np.float32))
    v = np.ascontiguousarray(np.asarray(e_vec, np.float32))
    b = np.ascontiguousarray(np.asarray(bias, np.float32))

    if "nc" not in _cache:
        _cache["nc"] = build(NT)
    nc = _cache["nc"]

    in_maps = []
    for i in range(NCORES):
        xf, lf = shard_inputs(x[i * NPER:(i + 1) * NPER], lab[i * NPER:(i + 1) * NPER])
        in_maps.append({"x": xf, "lab": lf, "w": Wm, "v": v, "b": b})

    res = run_bass_kernel_spmd(nc, in_maps, core_ids=list(range(NCORES)))
    LAST_RESULT = res
    outs = [unshard_output(res.results[i]["out"]) for i in range(NCORES)]
    return np.ascontiguousarray(np.concatenate(outs, axis=0))


# revision 43
# speedup vs baseline: 1.0063x; 1.0063x over previous
"""Trainium2 Bass kernel: GAT-style attention layer, data-parallel over 8 NeuronCores.

Reference computation (per node n, K=32 neighbors, D=128 features, L=64 labels):
    h     = lrelu(x @ W)                  [N,K,D]
    e     = lrelu(h @ v + bias)           [N,K,1]
    alpha = softmax_k(e)                  [N,K]
    out   = sum_k alpha[n,k] * labels[n,k,:]   [N,L]

Sharding: pure data parallel over nodes (6250/core, zero-padded to 6400).
Host side re-lays each core's shard out so every DMA is contiguous per
partition; values are untouched (fp32 in DRAM).

Device pipeline per 256-node tile (software-pipelined by one tile so the
TensorE never idles and its HAM clock stays at 2.4 GHz):
  mm1   z^T[e,(k,n)] = W^T @ x^T          TensorE bf16, PSUM f32
  lrelu PSUM->SBUF bf16 (ScalarE Prelu, fused activation)
  mm2   s[k,n] = v^T @ h^T via selector weights, PSUM-accumulated rows
  e=lrelu(s+bias), w=exp(e)               ScalarE (bias is per-partition AP)
  w^T   TensorE transpose -> [n,k]; row sums via ScalarE accum_out
  alpha = w^T * (1/sums)                  VectorE (per-partition scalar)
  diag(alpha_k) tiles on VectorE (identity mask * scalar)
  agg   out^T = sum_k labels_k^T-as-stationary @ diag(alpha_k)
        PSUM-accumulated; interleaved into the NEXT tile's mm1 stream
"""
import sys

sys.path.insert(0, "/opt/trn_rl_repo")
import numpy as np

N, K, D, L = 50000, 32, 128, 64
NEG = 0.2
NCORES = 8
NPER = N // NCORES          # 6250
TN = 256                    # nodes per tile
NSUB = TN // 128            # sub-tiles of 128 nodes
NPAD = 6400                 # padded nodes per core
NT = NPAD // TN             # 25 tiles

LAST_RESULT = None
_cache = {}


def build(nt):
    import concourse.bass as bass
    import concourse.tile as tile
    from concourse import bacc, mybir

    f32 = mybir.dt.float32
    bf16 = mybir.dt.bfloat16
    AF = mybir.ActivationFunctionType
    OP = mybir.AluOpType
    PSUM = bass.MemorySpace.PSUM

    nc = bacc.Bacc(
        "TRN2", target_bir_lowering=False, debug=False, num_devices=NCORES
    )
    x_ext = nc.declare_dram_parameter("x", [nt, 128, K * TN], f32, False)
    lab_ext = nc.declare_dram_parameter("lab", [nt, 128, NSUB * K * L], f32, False)
    w_ext = nc.declare_dram_parameter("w", [D, D], f32, False)
    v_ext = nc.declare_dram_parameter("v", [D, 1], f32, False)
    b_ext = nc.declare_dram_parameter("b", [K, 1], f32, False)
    out_ext = nc.declare_dram_parameter("out", [nt, 128, NSUB * L], f32, isOutput=True)

    with tile.TileContext(nc) as tc:
        with (
            tc.tile_pool(name="const", bufs=1) as const,
            tc.tile_pool(name="xp", bufs=3) as xp,
            tc.tile_pool(name="labp", bufs=3) as labp,
            tc.tile_pool(name="hp", bufs=2) as hp,
            tc.tile_pool(name="wp", bufs=2) as wp,
            tc.tile_pool(name="smallp", bufs=4) as smallp,
            tc.tile_pool(name="dkp", bufs=2) as dkp,
            tc.tile_pool(name="outp", bufs=2) as outp,
            tc.tile_pool(name="zps", bufs=2, space=PSUM) as zps,
            tc.tile_pool(name="sps", bufs=2, space=PSUM) as sps,
            tc.tile_pool(name="wtps", bufs=1, space=PSUM) as wtps,
        ):
            W_sb = const.tile([128, 128], bf16)
            nc.gpsimd.dma_start(W_sb[:], w_ext[:])      # f32 -> bf16 cast DMA
            v_sb = const.tile([128, 1], bf16)
            nc.gpsimd.dma_start(v_sb[:], v_ext[:])
            bias_sb = const.tile([32, 1], f32)
            nc.sync.dma_start(bias_sb[:], b_ext[:])
            ones = const.tile([128, 128], bf16)
            nc.vector.memset(ones[:], 1.0)
            mask = const.tile([128, 128], bf16)         # identity matrix
            nc.gpsimd.affine_select(
                mask[:], ones[:], pattern=[[1, 128]],
                compare_op=OP.is_equal, fill=0.0, base=0, channel_multiplier=-1,
            )
            # vks[:, 32k+m] = v * (m == k): selector weights so score matmul k
            # writes only PSUM row k of a [32, TN] tile (base partition stays 0)
            vks = const.tile([128, K * 32], bf16)
            nc.vector.memset(vks[:], 0.0)
            nc.vector.tensor_copy(
                vks[:, 0:K * 32:33], v_sb[:, 0:1].broadcast_to([128, 32])
            )

            # PE warmup burst: dense dummy matmuls while the first x tile
            # loads, so the HAM clock gate opens before real work starts
            warm_ps = zps.tile([128, 512], f32, name="warm_ps", tag="warm", bufs=1)
            for _ in range(20):
                nc.tensor.matmul(
                    warm_ps[:], W_sb[:], vks[:, 0:512], skip_group_check=True
                )

            nchunk = (K * TN) // 1024    # 8 mm1 chunks per tile (2 matmuls each)
            prev = None                  # state of tile t-1 awaiting aggregation

            def emit_softmax_tail(st):
                """TensorE transpose of exp-weights to [node, k], row sums via
                ScalarE accum_out, then normalized alpha (bf16) per sub-tile."""
                w_sb = st["w_sb"]
                st["alphaN"] = []
                for s in range(NSUB):
                    wT_ps = wtps.tile([128, 32], bf16)
                    nc.tensor.transpose(
                        wT_ps[:], w_sb[:, s * 128:(s + 1) * 128], mask[0:32, 0:32]
                    )
                    wT_sb = smallp.tile([128, 32], f32)
                    sums = smallp.tile([128, 1], f32)
                    nc.scalar.activation(wT_sb[:], wT_ps[:], AF.Copy, accum_out=sums[:])
                    recip = smallp.tile([128, 1], f32)
                    nc.vector.reciprocal(recip[:], sums[:])
                    alphaN = smallp.tile([128, 32], bf16, name=f"alphaN{s}", tag=f"al{s}")
                    nc.vector.tensor_scalar_mul(alphaN[:], wT_sb[:], recip[:, 0:1])
                    st["alphaN"].append(alphaN)
                st["out_sb"] = outp.tile([128, NSUB * L], f32, name="out_sb", tag="out")

            def emit_agg(st, s):
                """Weighted label aggregation for sub-tile s of tile st on
                VectorE: prod = labels * alpha (broadcast over l), then a
                strided reduction over k."""
                lab3 = st["lab_sb"][:, s * K * L:(s + 1) * K * L].rearrange(
                    "p (k l) -> p k l", k=K
                )
                al3 = st["alphaN"][s][:, 0:K].rearrange(
                    "p (k o) -> p k o", o=1
                ).broadcast_to([128, K, L])
                prod = dkp.tile([128, K * L], bf16, name=f"prod{s}", tag=f"prod{s}")
                nc.vector.tensor_tensor(
                    prod[:].rearrange("p (k l) -> p k l", k=K), lab3, al3, OP.mult
                )
                nc.vector.tensor_reduce(
                    st["out_sb"][:, s * L:(s + 1) * L],
                    prod[:].rearrange("p (k l) -> p l k", k=K),
                    op=OP.add, axis=mybir.AxisListType.X,
                )

            def emit_agg_finish(st):
                nc.sync.dma_start(out_ext[st["t"]], st["out_sb"][:])

            for t in range(nt):
                x_sb = xp.tile([128, K * TN], bf16)
                half = K * TN // 2
                nc.gpsimd.dma_start(x_sb[:, 0:half], x_ext[t][:, 0:half])
                nc.gpsimd.dma_start(x_sb[:, half:], x_ext[t][:, half:])
                # labels stay f32 and ride the HWDGE path: keeps the big cast
                # loads (x) alone on SWDGE, easing Q7 descriptor-ring pressure
                lab_sb = labp.tile([128, NSUB * K * L], f32)
                lhalf = NSUB * K * L // 2
                nc.sync.dma_start(lab_sb[:, 0:lhalf], lab_ext[t][:, 0:lhalf])
                nc.sync.dma_start(lab_sb[:, lhalf:], lab_ext[t][:, lhalf:])

                h_sb = hp.tile([128, K * TN], bf16)
                # two interleaved score-accumulation chains on alternating
                # PSUM banks (avoids same-bank drain turnaround between
                # consecutive accumulating matmuls)
                s_psA = sps.tile([32, TN], f32, name="s_psA", tag="spsA", bufs=1)
                s_psB = sps.tile([32, TN], f32, name="s_psB", tag="spsB", bufs=1)

                def emit_mm2(k):
                    tgt = s_psA if k % 2 == 0 else s_psB
                    nc.tensor.matmul(
                        tgt[:], vks[:, k * 32:(k + 1) * 32],
                        h_sb[:, k * TN:(k + 1) * TN],
                        start=(k < 2), stop=(k >= K - 2),
                    )

                # chunk 0, then the previous tile's softmax tail (transpose on
                # PE + diag builds on DVE) so its agg matmuls are ready to
                # interleave into the remaining chunks. mm2 for chunk c runs
                # one chunk late so the ScalarE lrelu latency is hidden.
                def emit_chunk(c):
                    # 1024-col chunk: two 512-col matmuls + one big lrelu
                    # (amortizes ScalarE's fixed per-op cost)
                    z_ps = zps.tile([128, 1024], f32, name="z_ps", tag="z")
                    nc.tensor.matmul(
                        z_ps[:, 0:512], W_sb[:], x_sb[:, c * 1024:c * 1024 + 512]
                    )
                    nc.tensor.matmul(
                        z_ps[:, 512:1024], W_sb[:], x_sb[:, c * 1024 + 512:(c + 1) * 1024]
                    )
                    nc.scalar.activation(
                        h_sb[:, c * 1024:(c + 1) * 1024], z_ps[:], AF.Prelu, alpha=NEG
                    )

                emit_chunk(0)
                if prev is not None:
                    emit_softmax_tail(prev)
                for c in range(1, nchunk):
                    emit_chunk(c)
                    if prev is not None and c == 2:
                        emit_agg(prev, 0)
                    if prev is not None and c == 5:
                        emit_agg(prev, 1)
                if prev is not None:
                    emit_agg_finish(prev)
                for k in range(K):
                    emit_mm2(k)

                s_tmp = wp.tile([32, TN], f32)
                nc.vector.tensor_copy(s_tmp[:], s_psA[:])
                s_sum = wp.tile([32, TN], f32)
                nc.vector.tensor_add(s_sum[:], s_tmp[:], s_psB[:])
                e_sb = wp.tile([32, TN], f32)
                nc.scalar.activation(
                    e_sb[:], s_sum[:], AF.Prelu, bias=bias_sb[:, 0:1], alpha=NEG
                )
                w_sb = wp.tile([32, TN], bf16)
                nc.scalar.activation(w_sb[:], e_sb[:], AF.Exp)

                prev = {"t": t, "w_sb": w_sb, "lab_sb": lab_sb}

            # drain the last tile
            emit_softmax_tail(prev)
            emit_agg(prev, 0)
            emit_agg(prev, 1)
            emit_agg_finish(prev)
    nc.compile()
    return nc


def shard_inputs(x, lab, nt=NT, nper=NPER):
    npad = nt * TN
    xs = np.zeros((npad, K, D), np.float32)
    xs[:nper] = x
    ls = np.zeros((npad, K, L), np.float32)
    ls[:nper] = lab
    xf = np.ascontiguousarray(
        xs.reshape(nt, TN, K, D).transpose(0, 3, 2, 1)
    ).reshape(nt, 128, K * TN)
    lf = np.ascontiguousarray(
        ls.reshape(nt, NSUB, 128, K * L).transpose(0, 2, 1, 3)
    ).reshape(nt, 128, NSUB * K * L)
    return xf, lf


def unshard_output(o, nt=NT, nper=NPER):
    # o[t, p, s*L + l] = pred[node = t*TN + s*128 + p, l]
    return (
        o.reshape(nt, 128, NSUB, L).transpose(0, 2, 1, 3).reshape(nt * TN, L)[:nper]
    )


def kernel(para_neighbors, para_nei_labels, linear, e_vec, bias):
    from concourse.bass_utils import run_bass_kernel_spmd

    global LAST_RESULT
    x = np.asarray(para_neighbors, np.float32)
    lab = np.asarray(para_nei_labels, np.float32)
    Wm = np.ascontiguousarray(np.asarray(linear, np.float32))
    v = np.ascontiguousarray(np.asarray(e_vec, np.float32))
    b = np.ascontiguousarray(np.asarray(bias, np.float32))

    if "nc" not in _cache:
        _cache["nc"] = build(NT)
    nc = _cache["nc"]

    in_maps = []
    for i in range(NCORES):
        xf, lf = shard_inputs(x[i * NPER:(i + 1) * NPER], lab[i * NPER:(i + 1) * NPER])
        in_maps.append({"x": xf, "lab": lf, "w": Wm, "v": v, "b": b})

    res = run_bass_kernel_spmd(nc, in_maps, core_ids=list(range(NCORES)))
    LAST_RESULT = res
    outs = [unshard_output(res.results[i]["out"]) for i in range(NCORES)]
    return np.ascontiguousarray(np.concatenate(outs, axis=0))
